# revision 1
# baseline (speedup 1.0000x reference)
"""ClassBalancedSupConLoss on 8 TRN2 NeuronCores (Bass/Tile).

Sharding: the BANK is column-sharded across the 8 cores (2048 cols each,
class-balanced with uniform cut positions), every core holds the full
(class-sorted) batch as matmul columns plus its own 256 anchors (merged
into one [anchors | gT | emb] input tensor). Each core computes, for
ALL 2048 anchors, exp-sums against its bank slice, and for its OWN
anchors the batch (bb) exp-sum total, self term, and positives
row-sums. The host (numpy, fp64) assembles denominators, logs, and the
masked mean.

Work split per [128, 2048] chunk (18 per core, one per anchor tile):
  PE : 4 x [128, 512] matmuls into a rotating PSUM tile (fp8 inputs)
  ACT: one Exp pass with accum_out = chunk total (free during ACTIVATE)
  DVE: own-class-range segment reduce of the bf16 exp output
The stream is ACT-bound (exp = 1 elem/cycle/lane at a fixed 1.2 GHz;
~36.9k exps per lane per core), so everything else hides behind it:
 - pure anchor tiles (all 128 sorted anchors share a class, 14 of 16):
   denominator = accum total - one own-class DVE reduce
 - class-straddling tiles (2): accum total + segments s0, s1 on the
   DVE; the host derives s2 = T - s0 - s1
 - bb chunks: no reduce at all: den_bb = total - selfe, where selfe =
   exp(inv_t*(s_ii-1)) is computed from the same rounded operands so
   the ~1.0 self term cancels at ACT-internal precision
 - keep the DVE LIGHT: measured on this part, any extra DVE traffic
   (bigger reduces, Schraudolph-offloaded exps, fatter scr pools)
   slows the ACT stream via SBUF/scheduling contention and loses more
   than it saves (knobs SUPCON_OFFLOAD / SUPCON_WARMUP / SUPCON_BB0SPLIT
   keep those failed variants testable).

Chunk order (measured): the diag/selfe/raw3 prelude runs FIRST so its
ACT work lands in the otherwise-idle DMA-wait window; bb0 opens the
stream (needs only the first input push); bank tiles 0..15 follow; bb1
closes it because it has no trailing DVE reduce, so the output DMA
fires straight after its accumulator read.

Class balancing: per class c every core gets exactly q_c =
2*floor(mcnt_c/16) bank columns; the per-core shortfall (2048 - sum q)
is zero-vector dummy columns whose exact exp(-inv_t) contribution the
host subtracts, and the <= 45 leftover real columns are folded in on
the host. Cut positions q0, q0+q1 are therefore compile-time constants
shared by all cores (SPMD-safe fixed-range reduces).

Numerics: matmul inputs fp8 e4m3 (logit noise washes out over the
18k-term sums; positives/self are consistent because host corrections
reuse the same quantized operands); exp outputs bf16; device sums fp32;
host assembly fp64. Measured rel err ~1.1e-3 (gate 2e-2).
"""

import os
import numpy as np

import concourse.bass as bass  # noqa: F401
from concourse import bacc
import concourse.mybir as mybir
import concourse.tile as tile
from concourse.bass_utils import run_bass_kernel_spmd

B, D, M, C = 2048, 128, 16384, 3
NCORES = 8
APC = B // NCORES          # own anchors per core = 256
NT = B // 128              # anchor tiles = 16 (all anchors)
NOWN = APC // 128          # own anchor tiles = 2
CH = 512                   # matmul free chunk (one PSUM bank)
W = 2048                   # chunk width = one PSUM [128, 2048] tile
BASE_TEMP = 0.07

F32 = mybir.dt.float32
BF16 = mybir.dt.bfloat16
AF = mybir.ActivationFunctionType
ALU = mybir.AluOpType
AX = mybir.AxisListType

# "f8"  : fp8 e4m3 matmul inputs (default)
# "bf16": bfloat16 matmul inputs (2x DMA bytes, less logit noise)
MM_MODE = os.environ.get("SUPCON_MM_MODE", "f8")
WARMUP = int(os.environ.get("SUPCON_WARMUP", "0"))
# number of pure bank chunks whose exp runs on the DVE (Schraudolph
# int16/bf16 bit-trick) instead of the saturated ACT engine
NOFF = int(os.environ.get("SUPCON_OFFLOAD", "0"))
BB0SPLIT = os.environ.get("SUPCON_BB0SPLIT", "0") == "1"

# bf16-domain Schraudolph exp: bitcast(int16(A16*y + B16)) ~= e^y
A16 = 128.0 / np.log(2.0)
B16_BASE = 127.0 * 128.0

LAST_EXEC_TIME_NS = None   # set by kernel() when SUPCON_TRACE=1

# oout column layout (per core, [128, OC] fp32)
OC_SEG = 0                 # 16 tiles x 3: pure = (T, own, -) / straddle = (s0, s1, s2)
OC_BBT = 48                # 2 own tiles: bb totals
OC_SELFE = 50              # 2
OC_RAW3 = 52               # 2 own tiles x 3 = 6
OC_SDIAG = 58              # 2
OC_BB0B = 60               # second half of the (split) first bb chunk
OC = 61


def _install_trace_shim():
    """Register the NTFF profile hook that this image's antenv lacks."""
    import sys
    import types
    import ctypes
    import contextlib

    try:
        from antenv.axon_hooks import get_axon_ntff_profile_hook  # noqa: F401
        return True  # real module exists
    except ImportError:
        pass

    so_path = "/opt/axon/libaxon_pjrt.so"
    if not os.path.exists(so_path):
        return False
    lib = ctypes.CDLL(so_path)
    if not hasattr(lib, "axon_start_nrt_profile"):
        return False
    lib.axon_start_nrt_profile.argtypes = [
        ctypes.POINTER(ctypes.c_int64),
        ctypes.c_size_t,
    ]
    lib.axon_start_nrt_profile.restype = ctypes.c_int64
    lib.axon_stop_nrt_profile.argtypes = [ctypes.c_char_p]
    lib.axon_stop_nrt_profile.restype = ctypes.c_int64

    @contextlib.contextmanager
    def _hook(output_dir, device_ids):
        import jax

        jax.devices()
        if device_ids:
            ids = (ctypes.c_int64 * len(device_ids))(*device_ids)
            rc = lib.axon_start_nrt_profile(ids, len(device_ids))
        else:
            rc = lib.axon_start_nrt_profile(None, 0)
        if rc != 0:
            raise RuntimeError(f"axon_start_nrt_profile rc={rc}")
        try:
            yield
        finally:
            n = lib.axon_stop_nrt_profile(str(output_dir).encode())
            print(f"profile: {n} file(s) written to {output_dir}", file=sys.stderr)

    _state = {"hook": _hook}
    mod = types.ModuleType("antenv.axon_hooks")
    mod.get_axon_ntff_profile_hook = lambda: _state["hook"]
    mod.set_axon_ntff_profile_hook = lambda h: _state.update(hook=h)
    sys.modules["antenv.axon_hooks"] = mod
    import antenv

    antenv.axon_hooks = mod

    import concourse.bass_utils as bu

    bu.upload_artifacts = lambda tmpdir: tmpdir
    return True


def _build(c1, c2, tile_cls, off_tiles, mm_mode):
    """c1/c2: class cuts in every core's bank slice; tile_cls[t]: class of
    anchor tile t if pure, else None (straddles a batch class boundary);
    off_tiles: pure bank tiles whose exp runs on the DVE (bit-trick)."""
    import ml_dtypes  # noqa: F401

    in_dt = mybir.dt.float8e4 if mm_mode == "f8" else BF16
    I16 = mybir.dt.int16

    AW = APC + 8               # anchor block: 256 own + 3 gT + 5 pad
    IW = AW + W                # one merged input: [anchors | batch emb]
    # invt | ninvt | invt_own | ninvt_own | eye  (+ sA | sB with offload)
    NV = (4 if off_tiles else 2) * NT + 2 * NOWN + 128

    nc = bacc.Bacc()
    inp_d = nc.declare_dram_parameter("inp", [D, IW], in_dt, isOutput=False)
    bankT_d = nc.declare_dram_parameter("bankT", [D, W], in_dt, isOutput=False)
    vecs_d = nc.declare_dram_parameter("vecs", [128, NV], F32, isOutput=False)
    oout_d = nc.declare_dram_parameter("oout", [128, OC], F32, isOutput=True)

    seg_r = [(0, c1), (c1, c2), (c2, W)]

    with tile.TileContext(nc) as tc:
        with (
            tc.tile_pool(name="big", bufs=1) as bigp,
            tc.tile_pool(name="sm", bufs=1) as smp,
            tc.tile_pool(name="scr", bufs=2) as scrp,
            tc.tile_pool(name="i16", bufs=2) as i16p,
            tc.tile_pool(name="ps", bufs=2, space="PSUM") as psp,
        ):
            inp_t = bigp.tile([D, IW], in_dt, tag="inp")
            bank_t = bigp.tile([D, W], in_dt, tag="bankT")

            def own(t):
                return inp_t[:, t * 128:(t + 1) * 128]

            def ecol(a, b):
                return inp_t[:, AW + a:AW + b]
            vecs_t = smp.tile([128, NV], F32, tag="vecs")
            o = [0]

            def vslice(w):
                a = o[0]; o[0] += w
                return vecs_t[:, a:a + w]
            invt_t = vslice(NT)
            ninvt_t = vslice(NT)
            invo_t = vslice(NOWN)
            ninvo_t = vslice(NOWN)
            if off_tiles:
                sA_t = vslice(NT)
                sB_t = vslice(NT)
            eye_t = vslice(128)
            junkx_t = bigp.tile([128, CH], in_dt, tag="junkx")

            oout_t = smp.tile([128, OC], F32, tag="oout")
            eyemul = smp.tile([128, 128], F32, tag="eyemul")
            warm = smp.tile([128, 1], F32, tag="warm")
            sdiag = [smp.tile([128, 1], F32, tag=f"sdiag{t}", name=f"sdiag{t}")
                     for t in range(NOWN)]

            # ACT first: exp table load + warm activation on junk data, so
            # the ~2.7us table load runs during the input DMA window.
            nc.vector.memset(junkx_t[:, 0:1], 0.0)
            nc.scalar.activation(warm[:], junkx_t[:, 0:1], AF.Exp)

            # input DMA: sync queue carries the merged [anchors | emb]
            # input in two pushes, scalar queue carries vecs + bank.
            # single inp push: 2312 B/partition lines stay above the ~2KB
            # threshold for full DMA throughput (sub-2KB column splits and
            # partition splits both measured slower)
            H = W // 2
            nc.scalar.dma_start(out=vecs_t[:], in_=vecs_d[:])
            nc.sync.dma_start(out=inp_t[:], in_=inp_d[:])
            nc.scalar.dma_start(out=bank_t[:], in_=bankT_d[:])

            # PE warmup on garbage operands (HAM clock-gate opener)
            if WARMUP:
                junkw_t = bigp.tile([128, 128], in_dt, tag="junkw")
                nc.vector.memset(junkw_t[:], 0.0)
                nc.vector.memset(junkx_t[:], 0.0)
                warm_ps = psp.tile([128, W], F32, tag="chunk", name="warm_ps")
                for w in range(WARMUP):
                    nc.tensor.matmul(
                        warm_ps[:, (w % 4) * CH:((w % 4) + 1) * CH],
                        junkw_t[:], junkx_t[:], start=True, stop=True,
                    )

            def ts_sum(src, width, col):
                """oout[col] = sum(src) on the DVE (TENSOR_REDUCE, 1x)."""
                nc.vector.reduce_sum(oout_t[:, col:col + 1], src, axis=AX.X)

            def emit_chunk(lhs, mtile, moff, sc, bi, accum, reduces, off=None):
                """[128, 2048] chunk: 4 matmuls + exp + range sums.

                ACT path: Exp (scale sc / bias bi) with optional accum col,
                then DVE range sums of the bf16 output.
                DVE path (off=(sa, sb)): Schraudolph int16 convert + bitcast
                bf16 range sums; ACT untouched."""
                ps = psp.tile([128, W], F32, tag="chunk", name="ps")
                for q in range(W // CH):
                    nc.tensor.matmul(
                        ps[:, q * CH:(q + 1) * CH], lhs,
                        mtile[:, moff + q * CH:moff + (q + 1) * CH],
                        start=True, stop=True,
                    )
                if off is not None:
                    # Schraudolph path: convert + reduce ONLY the listed
                    # ranges (kept under the 2-chunk PSUM cadence so the
                    # PE never stalls on this chunk's psum tile)
                    sa, sb = off
                    i16 = i16p.tile([128, W], I16, tag="i16", name="i16")
                    for (a, b, col) in reduces:
                        nc.vector.tensor_scalar(
                            out=i16[:, a:b], in0=ps[:, a:b], scalar1=sa,
                            scalar2=sb, op0=ALU.mult, op1=ALU.add)
                        ts_sum(i16[:, a:b].bitcast(BF16), b - a, col)
                else:
                    scr = scrp.tile([128, W], BF16, tag="scr", name="scr")
                    if isinstance(accum, list):
                        # split exp: one call (and accumulator) per range,
                        # so the first call starts before the full chunk's
                        # matmuls finish (head-latency trim)
                        for (a, b, acol) in accum:
                            nc.scalar.activation(
                                scr[:, a:b], ps[:, a:b], AF.Exp,
                                bias=bi, scale=sc,
                                accum_out=oout_t[:, acol:acol + 1])
                    else:
                        kw = {}
                        if accum is not None:
                            kw["accum_out"] = oout_t[:, accum:accum + 1]
                        nc.scalar.activation(
                            scr[:], ps[:], AF.Exp, bias=bi, scale=sc, **kw)
                    for (a, b, col) in reduces:
                        ts_sum(scr[:, a:b], b - a, col)

            # prelude: self-similarity diag + positives row-sums.  Runs
            # first so the selfe exps land in ACT's otherwise-idle DMA-wait
            # window; its DVE consumers finish before bb1 needs the slot
            post_ps = psp.tile([128, W], F32, tag="chunk", name="post_ps")
            for t in range(NOWN):
                nc.tensor.matmul(
                    post_ps[:, t * 128:(t + 1) * 128], own(t), own(t),
                    start=True, stop=True,
                )
            for t in range(NOWN):
                nc.tensor.matmul(
                    post_ps[:, 256 + t * C:256 + (t + 1) * C], own(t),
                    inp_t[:, APC:APC + C], start=True, stop=True,
                )
            for t in range(NOWN):
                nc.vector.tensor_mul(
                    eyemul[:], post_ps[:, t * 128:(t + 1) * 128], eye_t[:])
                nc.vector.reduce_sum(sdiag[t][:], eyemul[:], axis=AX.X)
                nc.scalar.activation(
                    oout_t[:, OC_SELFE + t:OC_SELFE + t + 1], sdiag[t][:],
                    AF.Exp, bias=ninvo_t[:, t:t + 1], scale=invo_t[:, t:t + 1],
                )
                nc.vector.tensor_copy(
                    out=oout_t[:, OC_SDIAG + t:OC_SDIAG + t + 1], in_=sdiag[t][:])
            nc.vector.tensor_copy(
                out=oout_t[:, OC_RAW3:OC_RAW3 + NOWN * C],
                in_=post_ps[:, 256:256 + NOWN * C])

            # bb chunks for the 2 own tiles: total only (host does T - selfe);
            # the very first exp is split in half so ACT starts sooner
            if BB0SPLIT:
                emit_chunk(own(0), inp_t, AW, invo_t[:, 0:1], ninvo_t[:, 0:1],
                           [(0, H, OC_BBT + 0), (H, W, OC_BB0B)], [])
            else:
                emit_chunk(own(0), inp_t, AW, invo_t[:, 0:1], ninvo_t[:, 0:1],
                           OC_BBT + 0, [])

            # bank chunks for all 16 anchor tiles.  Pure tiles: ACT accum
            # gives the chunk total, one DVE reduce gives the own-class
            # segment (host: den += T - own).  Offloaded pure class-0/2
            # tiles skip ACT entirely: the DVE bit-trick exps + sums just
            # the complement range (= the denominator part directly).
            for t in range(NT):
                lhs = ecol(t * 128, (t + 1) * 128)
                sc, bi = invt_t[:, t:t + 1], ninvt_t[:, t:t + 1]
                if t in off_tiles:
                    ct = tile_cls[t]
                    rng = (c1, W) if ct == 0 else (0, c2)
                    emit_chunk(lhs, bank_t, 0, sc, bi, None,
                               [(rng[0], rng[1], OC_SEG + t * 3 + 1)],
                               off=(sA_t[:, t:t + 1], sB_t[:, t:t + 1]))
                elif tile_cls[t] is not None:
                    a, bnd = seg_r[tile_cls[t]]
                    rd = ([(a, bnd, OC_SEG + t * 3 + 1)] if bnd > a else [])
                    emit_chunk(lhs, bank_t, 0, sc, bi, OC_SEG + t * 3, rd)
                else:
                    # straddle: ACT accum total + the first two segment
                    # sums on the DVE; host derives s2 = T - s0 - s1
                    rd = [(a, bnd, OC_SEG + t * 3 + ci)
                          for ci, (a, bnd) in enumerate(seg_r[:2]) if bnd > a]
                    emit_chunk(lhs, bank_t, 0, sc, bi, OC_SEG + t * 3 + 2, rd)

            # bb1 last: no DVE reduce trails it, so the output DMA fires
            # right after its accumulator read
            emit_chunk(own(1), inp_t, AW, invo_t[:, 1:2], ninvo_t[:, 1:2],
                       OC_BBT + 1, [])

            nc.sync.dma_start(out=oout_d[:], in_=oout_t[:])

    nc.compile()
    return nc


def kernel(embeddings, labels, bank_embs, bank_labels, class_temps):
    global LAST_EXEC_TIME_NS
    import ml_dtypes

    f8 = ml_dtypes.float8_e4m3
    in_np = f8 if MM_MODE == "f8" else ml_dtypes.bfloat16

    emb = np.asarray(embeddings, dtype=np.float32)
    bank = np.asarray(bank_embs, dtype=np.float32)
    lab = np.asarray(labels).astype(np.int64).ravel()
    blab = np.asarray(bank_labels).astype(np.int64).ravel()
    ct = np.asarray(class_temps, dtype=np.float32).ravel()

    # sort batch and bank by class
    bord = np.argsort(lab, kind="stable")
    slab = lab[bord]
    emb_s = emb[bord]                                  # [B, D] f32, sorted
    cnt = np.bincount(lab, minlength=C)
    mord = np.argsort(blab, kind="stable")
    bank_s = bank[mord]
    mcnt = np.bincount(blab, minlength=C)

    # per-core class quotas (even, for 4B-aligned bf16 reduce ranges)
    q = ((mcnt // NCORES) // 2 * 2).astype(np.int64)   # [3]
    sdum = int(W - q.sum())                            # zero-dummy cols/core
    assert sdum >= 0
    c1, c2 = int(q[0]), int(q[0] + q[1])
    cls_off = np.concatenate([[0], np.cumsum(mcnt)[:-1]])

    # anchor-tile purity (compile-time, same for all cores)
    tile_cls = []
    for t in range(NT):
        c_lo, c_hi = slab[t * 128], slab[t * 128 + 127]
        tile_cls.append(int(c_lo) if c_lo == c_hi else None)

    # quantized operands (shared by device and host-side corrections)
    embq = emb_s.astype(in_np)                         # [B, D]
    bankq = bank_s.astype(in_np)
    embq_f = embq.astype(np.float32)
    bankq_f = bankq.astype(np.float32)
    g = np.stack([emb_s[slab == c].sum(axis=0) for c in range(C)], axis=1)
    gq = g.astype(in_np)                               # [D, 3]

    inv_t_all = (1.0 / ct[slab]).astype(np.float32)    # [B] per sorted anchor

    # DVE-offloaded pure class-0/2 bank tiles, spread across the sequence
    pure02 = [t for t in range(NT) if tile_cls[t] in (0, 2)]
    noff = min(NOFF, len(pure02))
    off_tiles = (set(pure02[int(i)] for i in
                     np.linspace(0, len(pure02) - 1, noff).round())
                 if noff > 0 else set())

    # per-class Schraudolph bias tuning: pick corr_c that zeroes the mean
    # relative error of bitcast(int16(A16*y + B16 - corr)) over the y
    # distribution of this class's logits (s ~ N(0, 1/sqrt(D)))
    corr_cls = np.zeros(C)
    if off_tiles:
        sgrid = np.linspace(-4.0, 4.0, 4001) / np.sqrt(D)
        wpdf = np.exp(-0.5 * (sgrid * np.sqrt(D)) ** 2)
        for c in range(C):
            it = 1.0 / float(ct[c])
            y = it * (sgrid - 1.0)
            exact = np.exp(y)
            wexp = wpdf * exact
            best, bestv = 0.0, np.inf
            for corr in np.linspace(0.0, 12.0, 121):
                i16v = np.clip(np.rint(A16 * y + B16_BASE - corr), 0, 32767)
                approx = i16v.astype(np.int16).view(ml_dtypes.bfloat16).astype(np.float64)
                bias = abs(np.sum(wpdf * approx) / np.sum(wpdf * exact) - 1.0)
                if bias < bestv:
                    best, bestv = corr, bias
            corr_cls[c] = best

    nc = _build(c1, c2, tile_cls, off_tiles, MM_MODE)

    eye128 = np.eye(128, dtype=np.float32)
    embT = np.ascontiguousarray(embq.T)                # [D, B], shared
    invt_cols = np.ascontiguousarray(inv_t_all.reshape(NT, 128).T)
    sA_cols = (A16 * invt_cols).astype(np.float32)
    corr_all = corr_cls[slab]
    sB_all = (B16_BASE - corr_all - A16 * inv_t_all.astype(np.float64))
    sB_cols = np.ascontiguousarray(sB_all.reshape(NT, 128).T).astype(np.float32)
    AW = APC + 8
    in_maps = []
    for k in range(NCORES):
        asl = slice(k * APC, (k + 1) * APC)
        inp = np.zeros((D, AW + W), dtype=in_np)
        inp[:, 0:APC] = embq[asl].T
        inp[:, APC:APC + C] = gq
        inp[:, AW:AW + W] = embT
        bankT = np.zeros((D, W), dtype=in_np)
        pos = 0
        for c in range(C):
            sel = bankq[cls_off[c] + k * q[c]: cls_off[c] + (k + 1) * q[c]]
            bankT[:, pos:pos + q[c]] = sel.T
            pos += int(q[c])
        ivo = inv_t_all[asl]
        vparts = [
            invt_cols, -invt_cols,
            np.ascontiguousarray(ivo.reshape(NOWN, 128).T),
            np.ascontiguousarray((-ivo).reshape(NOWN, 128).T),
        ]
        if off_tiles:
            vparts += [sA_cols, sB_cols]
        vecs = np.concatenate(vparts + [eye128], axis=1).astype(np.float32)
        in_maps.append({
            "inp": np.ascontiguousarray(inp),
            "bankT": np.ascontiguousarray(bankT),
            "vecs": np.ascontiguousarray(vecs),
        })

    trace = os.environ.get("SUPCON_TRACE", "0") == "1"
    if trace:
        trace = _install_trace_shim()
    res = run_bass_kernel_spmd(nc, in_maps, core_ids=list(range(NCORES)), trace=trace)
    LAST_EXEC_TIME_NS = res.exec_time_ns

    # ---- host assembly (fp64) ----
    inv64 = inv_t_all.astype(np.float64)
    den = np.zeros(B, dtype=np.float64)
    raw3_own = np.zeros(B, dtype=np.float64)
    sdiag_own = np.zeros(B, dtype=np.float64)
    tidx = np.arange(128)
    for k in range(NCORES):
        oo = np.asarray(res.results[k]["oout"], dtype=np.float64)  # [128, OC]
        for t in range(NT):
            a_idx = t * 128 + tidx
            if t in off_tiles:
                den[a_idx] += oo[:, OC_SEG + t * 3 + 1]
            elif tile_cls[t] is not None:
                T = oo[:, OC_SEG + t * 3]
                own_s = (oo[:, OC_SEG + t * 3 + 1]
                         if q[tile_cls[t]] > 0 else 0.0)
                den[a_idx] += T - own_s
            else:
                # straddle tile: cols hold (s0, s1, T); s2 = T - s0 - s1
                s0 = oo[:, OC_SEG + t * 3 + 0] if c1 > 0 else 0.0
                s1 = oo[:, OC_SEG + t * 3 + 1] if c2 > c1 else 0.0
                s2 = oo[:, OC_SEG + t * 3 + 2] - s0 - s1
                segs = [s0, s1, s2]
                ca = slab[a_idx]
                for ci in range(C):
                    m = ca != ci
                    den[a_idx[m]] += (segs[ci][m] if np.ndim(segs[ci]) else 0.0)
        asl = slice(k * APC, (k + 1) * APC)
        for t in range(NOWN):
            a_idx = k * APC + t * 128 + tidx            # own anchors
            bbT = oo[:, OC_BBT + t]
            if t == 0 and BB0SPLIT:
                bbT = bbT + oo[:, OC_BB0B]
            den[a_idx] += bbT - oo[:, OC_SELFE + t]
            sdiag_own[a_idx] = oo[:, OC_SDIAG + t]
            cls = slab[a_idx]
            raw3_own[a_idx] = oo[tidx, OC_RAW3 + t * 3 + cls]

    # dummy correction: the sdum zero columns sit in the class-2 segment
    # (exp(-inv_t) each, per core); anchors of class 2 already exclude it
    if sdum > 0:
        m2 = slab != 2
        den[m2] -= NCORES * sdum * np.exp(-inv64[m2])

    # leftover (overflow) bank columns, folded in exactly on the host
    ov_cols, ov_cls = [], []
    for c in range(C):
        lo, hi = cls_off[c] + NCORES * q[c], cls_off[c] + mcnt[c]
        for j in range(lo, hi):
            ov_cols.append(j)
            ov_cls.append(c)
    if ov_cols:
        bq = bankq_f[ov_cols]                           # [n_ov, D]
        s_ov = embq_f @ bq.T                            # [B, n_ov]
        terms = np.exp(inv64[:, None] * (s_ov.astype(np.float64) - 1.0))
        mask = slab[:, None] != np.asarray(ov_cls)[None, :]
        den += (terms * mask).sum(axis=1)

    pos_cnt = (cnt[slab] - 1).astype(np.float64)
    pos_sum = raw3_own - sdiag_own
    pos_mean = pos_sum / np.maximum(pos_cnt, 1.0)
    log_denom = inv64 + np.log(den)
    coef = BASE_TEMP * inv64
    loss_i = coef * (log_denom - pos_mean)
    valid = pos_cnt > 0
    n_valid = int(valid.sum())
    loss = (loss_i * valid).sum() / max(n_valid, 1)
    return np.float32(loss)



# revision 7
# speedup vs baseline: 1.0809x; 1.0809x over previous
"""ClassBalancedSupConLoss on 8 TRN2 NeuronCores (Bass/Tile).

Sharding: the BANK is column-sharded across the 8 cores (2048 cols each,
class-balanced quotas q_c = floor(mcnt_c/8)), every core holds the full
(class-sorted) batch as matmul columns plus its own 256 anchors (merged
into one [anchors | gT | emb] input tensor).

Key idea vs the old kernel: the denominator only needs OTHER-class bank
terms, so per anchor tile we matmul + exp ONLY the complement-class
columns (~2/3 of the slice), packed contiguously in PSUM by slicing the
moving operand.  The ACT accumulator total of that range IS the bank
denominator contribution directly -- no segment reduces at all:

  pure tile (class c): psum <- [other segments packed], one Exp with
    accum_out = den col.  ACT cost ~1.37k elem/lane, zero DVE.
  straddle tile (classes a<b, split row r): psum <- [S_b | S_other |
    S_a]; compl(a) is the prefix, compl(b) the suffix; two
    partition-sliced Exps with separate accum cols.
  bb tiles: full 2048 batch cols, accum total; host does T - selfe
    with selfe computed from the same quantized diag (prelude).

Work split: NOFF of the pure tiles run on the DVE instead (Schraudolph
int16/bf16 bit-trick convert + one bf16 range-sum), placed at odd
emission slots so they rotate through the other PSUM buffer than the
ACT-consumed tiles.  Positives come from the g-trick (e_i . g_c) in the
prelude, as before.

Numerics: matmul inputs fp8 e4m3, exp outputs bf16 (ACT) / int16-bitcast
(DVE), device sums fp32, host assembly fp64.
"""

import os
import numpy as np

import concourse.bass as bass  # noqa: F401
from concourse import bacc
import concourse.mybir as mybir
import concourse.tile as tile
from concourse.bass_utils import run_bass_kernel_spmd

B, D, M, C = 2048, 128, 16384, 3
NCORES = 8
APC = B // NCORES          # own anchors per core = 256
NT = B // 128              # anchor tiles = 16 (all anchors)
NOWN = APC // 128          # own anchor tiles = 2
CH = 512                   # matmul free chunk (one PSUM bank)
W = 2048                   # bank slice cols per core
BASE_TEMP = 0.07

F32 = mybir.dt.float32
BF16 = mybir.dt.bfloat16
I16 = mybir.dt.int16
AF = mybir.ActivationFunctionType
ALU = mybir.AluOpType
AX = mybir.AxisListType

MM_MODE = os.environ.get("SUPCON_MM_MODE", "f8")
WARMUP = int(os.environ.get("SUPCON_WARMUP", "0"))
NOFF = int(os.environ.get("SUPCON_OFFLOAD", "0"))
BB0SPLIT = os.environ.get("SUPCON_BB0SPLIT", "0") == "1"

# bf16-domain Schraudolph exp: bitcast(int16(A16*y + B16)) ~= e^y
A16 = 128.0 / np.log(2.0)
B16_BASE = 127.0 * 128.0

LAST_EXEC_TIME_NS = None   # set by kernel() when SUPCON_TRACE=1

# oout column layout (per core, [128, OC] fp32)
OC_DEN = 0                 # 16: per-tile bank den contribution (or total T
                           #     for straddle tiles)
OC_DEN2 = 16               # 2 straddles x 2: S_b and S_a segment sums
OC_BBT = 20                # 2 own tiles: bb totals
OC_SELFE = 22              # 2
OC_RAW3 = 24               # 2 own tiles x 3 = 6
OC_SDIAG = 30              # 2
OC_BB0B = 32               # second half of the (split) first bb chunk
OC = 33


def _install_trace_shim():
    """Register the NTFF profile hook that this image's antenv lacks."""
    import sys
    import types
    import ctypes
    import contextlib

    try:
        from antenv.axon_hooks import get_axon_ntff_profile_hook  # noqa: F401
        return True  # real module exists
    except ImportError:
        pass

    so_path = "/opt/axon/libaxon_pjrt.so"
    if not os.path.exists(so_path):
        return False
    lib = ctypes.CDLL(so_path)
    if not hasattr(lib, "axon_start_nrt_profile"):
        return False
    lib.axon_start_nrt_profile.argtypes = [
        ctypes.POINTER(ctypes.c_int64),
        ctypes.c_size_t,
    ]
    lib.axon_start_nrt_profile.restype = ctypes.c_int64
    lib.axon_stop_nrt_profile.argtypes = [ctypes.c_char_p]
    lib.axon_stop_nrt_profile.restype = ctypes.c_int64

    @contextlib.contextmanager
    def _hook(output_dir, device_ids):
        import jax

        jax.devices()
        if device_ids:
            ids = (ctypes.c_int64 * len(device_ids))(*device_ids)
            rc = lib.axon_start_nrt_profile(ids, len(device_ids))
        else:
            rc = lib.axon_start_nrt_profile(None, 0)
        if rc != 0:
            raise RuntimeError(f"axon_start_nrt_profile rc={rc}")
        try:
            yield
        finally:
            n = lib.axon_stop_nrt_profile(str(output_dir).encode())
            print(f"profile: {n} file(s) written to {output_dir}", file=sys.stderr)

    _state = {"hook": _hook}
    mod = types.ModuleType("antenv.axon_hooks")
    mod.get_axon_ntff_profile_hook = lambda: _state["hook"]
    mod.set_axon_ntff_profile_hook = lambda h: _state.update(hook=h)
    sys.modules["antenv.axon_hooks"] = mod
    import antenv

    antenv.axon_hooks = mod

    import concourse.bass_utils as bu

    bu.upload_artifacts = lambda tmpdir: tmpdir
    return True


def _tile_plan(q, tile_cls, strad_r):
    """Per-tile PSUM packing plan.

    q: [3] per-core class quotas.  Returns for each tile t a dict with
      spans: list of (psum_off, bank_off, len) matmul segments, cut at
             PSUM 512 boundaries
      w: total packed width
      segs: for straddle tiles, the two (f0, f1) PSUM ranges whose bf16
            sums (S_b then S_a) the host subtracts from the full total;
            None for pure tiles
    """
    off = [0, int(q[0]), int(q[0] + q[1])]
    q = [int(x) for x in q]
    plans = []
    for t in range(NT):
        c = tile_cls[t]
        if c is not None:
            order = [x for x in range(C) if x != c and q[x] > 0]
        else:
            a, b = strad_r[t][0], strad_r[t][1]
            o = 3 - a - b
            order = [x for x in (b, o, a) if q[x] > 0]
        spans = []
        p = 0
        for x in order:
            boff, blen = off[x], q[x]
            s = 0
            while s < blen:
                take = min(blen - s, CH - (p % CH))
                spans.append((p, boff + s, take))
                p += take
                s += take
        w = p
        if c is not None:
            segs = None
        else:
            a, b = strad_r[t][0], strad_r[t][1]
            o = 3 - a - b
            # psum layout [S_b | S_o | S_a]
            segs = [(0, q[b]), (q[b] + q[o], w)]
        plans.append({"spans": spans, "w": w, "segs": segs})
    return plans


def _build(plans, off_tiles, emit_order, mm_mode):
    import ml_dtypes  # noqa: F401

    in_dt = mybir.dt.float8e4 if mm_mode == "f8" else BF16

    AW = APC + 8               # anchor block: 256 own + 3 gT + 5 pad
    IW = AW + W                # one merged input: [anchors | batch emb]
    NV = 4 * NT + 2 * NOWN + 128

    nc = bacc.Bacc()
    inp_d = nc.declare_dram_parameter("inp", [D, IW], in_dt, isOutput=False)
    bankT_d = nc.declare_dram_parameter("bankT", [D, W], in_dt, isOutput=False)
    vecs_d = nc.declare_dram_parameter("vecs", [128, NV], F32, isOutput=False)
    oout_d = nc.declare_dram_parameter("oout", [128, OC], F32, isOutput=True)

    with tile.TileContext(nc) as tc:
        with (
            tc.tile_pool(name="big", bufs=1) as bigp,
            tc.tile_pool(name="sm", bufs=1) as smp,
            tc.tile_pool(name="scr", bufs=2) as scrp,
            tc.tile_pool(name="i16", bufs=2) as i16p,
            tc.tile_pool(name="ps", bufs=2, space="PSUM") as psp,
        ):
            inp_t = bigp.tile([D, IW], in_dt, tag="inp")
            bank_t = bigp.tile([D, W], in_dt, tag="bankT")

            def own(t):
                return inp_t[:, t * 128:(t + 1) * 128]

            def ecol(a, b):
                return inp_t[:, AW + a:AW + b]
            vecs_t = smp.tile([128, NV], F32, tag="vecs")
            o = [0]

            def vslice(w):
                a = o[0]; o[0] += w
                return vecs_t[:, a:a + w]
            invt_t = vslice(NT)
            ninvt_t = vslice(NT)
            invo_t = vslice(NOWN)
            ninvo_t = vslice(NOWN)
            sA_t = vslice(NT)
            sB_t = vslice(NT)
            eye_t = vslice(128)
            junkx_t = bigp.tile([128, CH], in_dt, tag="junkx")

            oout_t = smp.tile([128, OC], F32, tag="oout")
            eyemul = smp.tile([128, 128], F32, tag="eyemul")
            warm = smp.tile([128, 1], F32, tag="warm")
            sdiag = [smp.tile([128, 1], F32, tag=f"sdiag{t}", name=f"sdiag{t}")
                     for t in range(NOWN)]

            # ACT first: exp table load + warm activation on junk data, so
            # the ~2.7us table load runs during the input DMA window.
            nc.vector.memset(junkx_t[:, 0:1], 0.0)
            nc.scalar.activation(warm[:], junkx_t[:, 0:1], AF.Exp)

            H = W // 2
            nc.scalar.dma_start(out=vecs_t[:], in_=vecs_d[:])
            nc.sync.dma_start(out=inp_t[:], in_=inp_d[:])
            nc.scalar.dma_start(out=bank_t[:], in_=bankT_d[:])

            # PE warmup on garbage operands (HAM clock-gate opener)
            if WARMUP:
                junkw_t = bigp.tile([128, 128], in_dt, tag="junkw")
                nc.vector.memset(junkw_t[:], 0.0)
                nc.vector.memset(junkx_t[:], 0.0)
                warm_ps = psp.tile([128, W], F32, tag="chunk", name="warm_ps")
                for w in range(WARMUP):
                    nc.tensor.matmul(
                        warm_ps[:, (w % 4) * CH:((w % 4) + 1) * CH],
                        junkw_t[:], junkx_t[:], start=True, stop=True,
                    )

            # prelude: self-similarity diag + positives row-sums.  Runs
            # first so the selfe exps land in ACT's otherwise-idle DMA-wait
            # window; its DVE consumers finish before bb1 needs the slot
            post_ps = psp.tile([128, W], F32, tag="chunk", name="post_ps")
            for t in range(NOWN):
                nc.tensor.matmul(
                    post_ps[:, t * 128:(t + 1) * 128], own(t), own(t),
                    start=True, stop=True,
                )
            for t in range(NOWN):
                nc.tensor.matmul(
                    post_ps[:, 256 + t * C:256 + (t + 1) * C], own(t),
                    inp_t[:, APC:APC + C], start=True, stop=True,
                )
            for t in range(NOWN):
                nc.vector.tensor_mul(
                    eyemul[:], post_ps[:, t * 128:(t + 1) * 128], eye_t[:])
                nc.vector.reduce_sum(sdiag[t][:], eyemul[:], axis=AX.X)
                nc.scalar.activation(
                    oout_t[:, OC_SELFE + t:OC_SELFE + t + 1], sdiag[t][:],
                    AF.Exp, bias=ninvo_t[:, t:t + 1], scale=invo_t[:, t:t + 1],
                )
                nc.vector.tensor_copy(
                    out=oout_t[:, OC_SDIAG + t:OC_SDIAG + t + 1], in_=sdiag[t][:])
            nc.vector.tensor_copy(
                out=oout_t[:, OC_RAW3:OC_RAW3 + NOWN * C],
                in_=post_ps[:, 256:256 + NOWN * C])

            def emit_bb(t, accum):
                """Full [128, 2048] bb chunk: 4 matmuls + exp w/ accum."""
                ps = psp.tile([128, W], F32, tag="chunk", name=f"bb{t}")
                for qq in range(W // CH):
                    nc.tensor.matmul(
                        ps[:, qq * CH:(qq + 1) * CH], own(t),
                        inp_t[:, AW + qq * CH:AW + (qq + 1) * CH],
                        start=True, stop=True,
                    )
                scr = scrp.tile([128, W], BF16, tag="scr", name=f"bbs{t}")
                sc, bi = invo_t[:, t:t + 1], ninvo_t[:, t:t + 1]
                if isinstance(accum, list):
                    for (a, b, acol) in accum:
                        nc.scalar.activation(
                            scr[:, a:b], ps[:, a:b], AF.Exp,
                            bias=bi, scale=sc,
                            accum_out=oout_t[:, acol:acol + 1])
                else:
                    nc.scalar.activation(
                        scr[:], ps[:], AF.Exp, bias=bi, scale=sc,
                        accum_out=oout_t[:, accum:accum + 1])

            def emit_bank(t, si):
                """Complement-packed bank chunk for anchor tile t.

                si: straddle index (0/1) if tile t straddles, else None.
                Pure ACT: one Exp over [0:w) with accum_out = den col.
                Straddle: Exp over the full packed width, accum = total T;
                  DVE sums the S_b / S_a bf16 ranges; host subtracts.
                DVE path: Schraudolph int16 convert + one bf16 range sum."""
                plan = plans[t]
                w = plan["w"]
                ps = psp.tile([128, W], F32, tag="chunk", name=f"ps{t}")
                lhs = ecol(t * 128, (t + 1) * 128)
                for (poff, boff, ln) in plan["spans"]:
                    nc.tensor.matmul(
                        ps[:, poff:poff + ln], lhs, bank_t[:, boff:boff + ln],
                        start=True, stop=True,
                    )
                if t in off_tiles:
                    i16 = i16p.tile([128, W], I16, tag="i16", name=f"i16{t}")
                    nc.vector.tensor_scalar(
                        out=i16[:, 0:w], in0=ps[:, 0:w],
                        scalar1=sA_t[:, t:t + 1], scalar2=sB_t[:, t:t + 1],
                        op0=ALU.mult, op1=ALU.add)
                    nc.vector.reduce_sum(
                        oout_t[:, OC_DEN + t:OC_DEN + t + 1],
                        i16[:, 0:w].bitcast(BF16), axis=AX.X)
                else:
                    scr = scrp.tile([128, W], BF16, tag="scr", name=f"scr{t}")
                    nc.scalar.activation(
                        scr[:, 0:w], ps[:, 0:w], AF.Exp,
                        bias=ninvt_t[:, t:t + 1], scale=invt_t[:, t:t + 1],
                        accum_out=oout_t[:, OC_DEN + t:OC_DEN + t + 1])
                    if plan["segs"] is not None:
                        for j, (f0, f1) in enumerate(plan["segs"]):
                            nc.vector.reduce_sum(
                                oout_t[:, OC_DEN2 + 2 * si + j:
                                       OC_DEN2 + 2 * si + j + 1],
                                scr[:, f0:f1], axis=AX.X)

            # bb0 opens the stream (needs only the first input push)
            if BB0SPLIT:
                emit_bb(0, [(0, H, OC_BBT + 0), (H, W, OC_BB0B)])
            else:
                emit_bb(0, OC_BBT + 0)

            # bank tiles in planned emission order (off tiles at odd slots)
            strad_seen = 0
            for t in emit_order:
                si = None
                if plans[t]["segs"] is not None:
                    si = strad_seen
                    strad_seen += 1
                emit_bank(t, si)

            # bb1 last: no trailing DVE reduce, so the output DMA fires
            # right after its accumulator read
            emit_bb(1, OC_BBT + 1)

            nc.sync.dma_start(out=oout_d[:], in_=oout_t[:])

    nc.compile()
    return nc


def kernel(embeddings, labels, bank_embs, bank_labels, class_temps):
    global LAST_EXEC_TIME_NS
    import ml_dtypes

    f8 = ml_dtypes.float8_e4m3
    in_np = f8 if MM_MODE == "f8" else ml_dtypes.bfloat16

    emb = np.asarray(embeddings, dtype=np.float32)
    bank = np.asarray(bank_embs, dtype=np.float32)
    lab = np.asarray(labels).astype(np.int64).ravel()
    blab = np.asarray(bank_labels).astype(np.int64).ravel()
    ct = np.asarray(class_temps, dtype=np.float32).ravel()

    # sort batch and bank by class
    bord = np.argsort(lab, kind="stable")
    slab = lab[bord]
    emb_s = emb[bord]                                  # [B, D] f32, sorted
    cnt = np.bincount(lab, minlength=C)
    mord = np.argsort(blab, kind="stable")
    bank_s = bank[mord]
    mcnt = np.bincount(blab, minlength=C)

    # per-core class quotas; <=7*C leftover cols folded in on the host
    q = (mcnt // NCORES).astype(np.int64)              # [3]
    assert int(q.sum()) <= W
    cls_off = np.concatenate([[0], np.cumsum(mcnt)[:-1]])

    # anchor-tile purity (compile-time, same for all cores)
    tile_cls = []
    strad_r = {}
    for t in range(NT):
        seg = slab[t * 128:(t + 1) * 128]
        c_lo, c_hi = int(seg[0]), int(seg[-1])
        if c_lo == c_hi:
            tile_cls.append(c_lo)
        else:
            tile_cls.append(None)
            r = int(np.searchsorted(seg, c_lo, side="right"))
            assert seg[r] == c_hi, "tile straddles >2 classes"
            strad_r[t] = (c_lo, c_hi, r)

    plans = _tile_plan(q, tile_cls, strad_r)

    # quantized operands (shared by device and host-side corrections)
    embq = emb_s.astype(in_np)                         # [B, D]
    bankq = bank_s.astype(in_np)
    embq_f = embq.astype(np.float32)
    bankq_f = bankq.astype(np.float32)
    g = np.stack([emb_s[slab == c].sum(axis=0) for c in range(C)], axis=1)
    gq = g.astype(in_np)                               # [D, 3]

    inv_t_all = (1.0 / ct[slab]).astype(np.float32)    # [B] per sorted anchor

    # DVE-offloaded pure tiles, interleaved at odd emission slots
    pure = [t for t in range(NT) if tile_cls[t] is not None]
    noff = min(NOFF, len(pure), NT // 2)
    off_tiles = (set(pure[int(i)] for i in
                     np.linspace(0, len(pure) - 1, noff).round())
                 if noff > 0 else set())
    on_list = [t for t in range(NT) if t not in off_tiles]
    off_list = [t for t in range(NT) if t in off_tiles]
    emit_order = []
    ia = ib = 0
    for j in range(NT):
        if j % 2 == 1 and ib < len(off_list):
            emit_order.append(off_list[ib]); ib += 1
        else:
            emit_order.append(on_list[ia]); ia += 1

    # per-class Schraudolph bias tuning: pick corr_c that zeroes the mean
    # relative error over the y distribution of this class's logits
    corr_cls = np.zeros(C)
    if off_tiles:
        sgrid = np.linspace(-4.0, 4.0, 4001) / np.sqrt(D)
        wpdf = np.exp(-0.5 * (sgrid * np.sqrt(D)) ** 2)
        for c in range(C):
            it = 1.0 / float(ct[c])
            y = it * (sgrid - 1.0)
            exact = np.exp(y)
            best, bestv = 0.0, np.inf
            for corr in np.linspace(0.0, 12.0, 121):
                i16v = np.clip(np.rint(A16 * y + B16_BASE - corr), 0, 32767)
                approx = i16v.astype(np.int16).view(ml_dtypes.bfloat16).astype(np.float64)
                bias = abs(np.sum(wpdf * approx) / np.sum(wpdf * exact) - 1.0)
                if bias < bestv:
                    best, bestv = corr, bias
            corr_cls[c] = best

    nc = _build(plans, off_tiles, emit_order, MM_MODE)

    eye128 = np.eye(128, dtype=np.float32)
    embT = np.ascontiguousarray(embq.T)                # [D, B], shared
    invt_cols = np.ascontiguousarray(inv_t_all.reshape(NT, 128).T)
    sA_cols = (A16 * invt_cols).astype(np.float32)
    corr_all = corr_cls[slab]
    sB_all = (B16_BASE - corr_all - A16 * inv_t_all.astype(np.float64))
    sB_cols = np.ascontiguousarray(sB_all.reshape(NT, 128).T).astype(np.float32)
    AW = APC + 8
    in_maps = []
    for k in range(NCORES):
        asl = slice(k * APC, (k + 1) * APC)
        inp = np.zeros((D, AW + W), dtype=in_np)
        inp[:, 0:APC] = embq[asl].T
        inp[:, APC:APC + C] = gq
        inp[:, AW:AW + W] = embT
        bankT = np.zeros((D, W), dtype=in_np)
        pos = 0
        for c in range(C):
            sel = bankq[cls_off[c] + k * q[c]: cls_off[c] + (k + 1) * q[c]]
            bankT[:, pos:pos + q[c]] = sel.T
            pos += int(q[c])
        ivo = inv_t_all[asl]
        vparts = [
            invt_cols, -invt_cols,
            np.ascontiguousarray(ivo.reshape(NOWN, 128).T),
            np.ascontiguousarray((-ivo).reshape(NOWN, 128).T),
            sA_cols, sB_cols,
        ]
        vecs = np.concatenate(vparts + [eye128], axis=1).astype(np.float32)
        in_maps.append({
            "inp": np.ascontiguousarray(inp),
            "bankT": np.ascontiguousarray(bankT),
            "vecs": np.ascontiguousarray(vecs),
        })

    trace = os.environ.get("SUPCON_TRACE", "0") == "1"
    if trace:
        trace = _install_trace_shim()
    res = run_bass_kernel_spmd(nc, in_maps, core_ids=list(range(NCORES)), trace=trace)
    LAST_EXEC_TIME_NS = res.exec_time_ns

    # ---- host assembly (fp64) ----
    inv64 = inv_t_all.astype(np.float64)
    den = np.zeros(B, dtype=np.float64)
    raw3_own = np.zeros(B, dtype=np.float64)
    sdiag_own = np.zeros(B, dtype=np.float64)
    tidx = np.arange(128)
    strad_order = sorted(strad_r.keys(), key=lambda t: emit_order.index(t))
    for k in range(NCORES):
        oo = np.asarray(res.results[k]["oout"], dtype=np.float64)  # [128, OC]
        for t in range(NT):
            a_idx = t * 128 + tidx
            if tile_cls[t] is not None or t in off_tiles:
                den[a_idx] += oo[:, OC_DEN + t]
            else:
                # rows [0:r) are class a: den = T - S_a; rows [r:) class b:
                # den = T - S_b (cols OC_DEN2+2si = S_b, +2si+1 = S_a)
                r = strad_r[t][2]
                si = strad_order.index(t)
                T = oo[:, OC_DEN + t]
                s_b = oo[:, OC_DEN2 + 2 * si]
                s_a = oo[:, OC_DEN2 + 2 * si + 1]
                den[a_idx[:r]] += T[:r] - s_a[:r]
                den[a_idx[r:]] += T[r:] - s_b[r:]
        for t in range(NOWN):
            a_idx = k * APC + t * 128 + tidx            # own anchors
            bbT = oo[:, OC_BBT + t]
            if t == 0 and BB0SPLIT:
                bbT = bbT + oo[:, OC_BB0B]
            den[a_idx] += bbT - oo[:, OC_SELFE + t]
            sdiag_own[a_idx] = oo[:, OC_SDIAG + t]
            cls = slab[a_idx]
            raw3_own[a_idx] = oo[tidx, OC_RAW3 + t * 3 + cls]

    # leftover (overflow) bank columns, folded in exactly on the host
    ov_cols, ov_cls = [], []
    for c in range(C):
        lo, hi = cls_off[c] + NCORES * q[c], cls_off[c] + mcnt[c]
        for j in range(lo, hi):
            ov_cols.append(j)
            ov_cls.append(c)
    if ov_cols:
        bq = bankq_f[ov_cols]                           # [n_ov, D]
        s_ov = embq_f @ bq.T                            # [B, n_ov]
        terms = np.exp(inv64[:, None] * (s_ov.astype(np.float64) - 1.0))
        mask = slab[:, None] != np.asarray(ov_cls)[None, :]
        den += (terms * mask).sum(axis=1)

    pos_cnt = (cnt[slab] - 1).astype(np.float64)
    pos_sum = raw3_own - sdiag_own
    pos_mean = pos_sum / np.maximum(pos_cnt, 1.0)
    log_denom = inv64 + np.log(den)
    coef = BASE_TEMP * inv64
    loss_i = coef * (log_denom - pos_mean)
    valid = pos_cnt > 0
    n_valid = int(valid.sum())
    loss = (loss_i * valid).sum() / max(n_valid, 1)
    return np.float32(loss)


# revision 8
# speedup vs baseline: 1.1281x; 1.0437x over previous
"""ClassBalancedSupConLoss on 8 TRN2 NeuronCores (Bass/Tile).

Sharding: the BANK is column-sharded across the 8 cores (2048 cols each,
class-balanced quotas q_c = floor(mcnt_c/8)), every core holds the full
(class-sorted) batch as matmul columns plus its own 256 anchors (merged
into one [anchors | gT | emb] input tensor).

Key idea vs the old kernel: the denominator only needs OTHER-class bank
terms, so per anchor tile we matmul + exp ONLY the complement-class
columns (~2/3 of the slice), packed contiguously in PSUM by slicing the
moving operand.  The ACT accumulator total of that range IS the bank
denominator contribution directly -- no segment reduces at all:

  pure tile (class c): psum <- [other segments packed], one Exp with
    accum_out = den col.  ACT cost ~1.37k elem/lane, zero DVE.
  straddle tile (classes a<b, split row r): psum <- [S_b | S_other |
    S_a]; compl(a) is the prefix, compl(b) the suffix; two
    partition-sliced Exps with separate accum cols.
  bb tiles: full 2048 batch cols, accum total; host does T - selfe
    with selfe computed from the same quantized diag (prelude).

Work split: NOFF of the pure tiles run on the DVE instead (Schraudolph
int16/bf16 bit-trick convert + one bf16 range-sum), placed at odd
emission slots so they rotate through the other PSUM buffer than the
ACT-consumed tiles.  Positives come from the g-trick (e_i . g_c) in the
prelude, as before.

Numerics: matmul inputs fp8 e4m3, exp outputs bf16 (ACT) / int16-bitcast
(DVE), device sums fp32, host assembly fp64.
"""

import os
import numpy as np

import concourse.bass as bass  # noqa: F401
from concourse import bacc
import concourse.mybir as mybir
import concourse.tile as tile
from concourse.bass_utils import run_bass_kernel_spmd

B, D, M, C = 2048, 128, 16384, 3
NCORES = 8
APC = B // NCORES          # own anchors per core = 256
NT = B // 128              # anchor tiles = 16 (all anchors)
NOWN = APC // 128          # own anchor tiles = 2
CH = 512                   # matmul free chunk (one PSUM bank)
W = 2048                   # bank slice cols per core
BASE_TEMP = 0.07

F32 = mybir.dt.float32
BF16 = mybir.dt.bfloat16
I16 = mybir.dt.int16
AF = mybir.ActivationFunctionType
ALU = mybir.AluOpType
AX = mybir.AxisListType

MM_MODE = os.environ.get("SUPCON_MM_MODE", "f8")
WARMUP = int(os.environ.get("SUPCON_WARMUP", "0"))
NOFF = int(os.environ.get("SUPCON_OFFLOAD", "0"))
BB0SPLIT = os.environ.get("SUPCON_BB0SPLIT", "0") == "1"

# bf16-domain Schraudolph exp: bitcast(int16(A16*y + B16)) ~= e^y
A16 = 128.0 / np.log(2.0)
B16_BASE = 127.0 * 128.0

LAST_EXEC_TIME_NS = None   # set by kernel() when SUPCON_TRACE=1

# oout column layout (per core, [128, OC] fp32)
OC_DEN = 0                 # 16: per-tile bank den contribution (or total T
                           #     for straddle tiles)
OC_DEN2 = 16               # 2 straddles x 2: S_b and S_a segment sums
OC_BBT = 20                # 2 own tiles: bb totals
OC_SELFE = 22              # 2
OC_RAW3 = 24               # 2 own tiles x 3 = 6
OC_SDIAG = 30              # 2
OC_BB0B = 32               # second half of the (split) first bb chunk
OC = 33


def _install_trace_shim():
    """Register the NTFF profile hook that this image's antenv lacks."""
    import sys
    import types
    import ctypes
    import contextlib

    try:
        from antenv.axon_hooks import get_axon_ntff_profile_hook  # noqa: F401
        return True  # real module exists
    except ImportError:
        pass

    so_path = "/opt/axon/libaxon_pjrt.so"
    if not os.path.exists(so_path):
        return False
    lib = ctypes.CDLL(so_path)
    if not hasattr(lib, "axon_start_nrt_profile"):
        return False
    lib.axon_start_nrt_profile.argtypes = [
        ctypes.POINTER(ctypes.c_int64),
        ctypes.c_size_t,
    ]
    lib.axon_start_nrt_profile.restype = ctypes.c_int64
    lib.axon_stop_nrt_profile.argtypes = [ctypes.c_char_p]
    lib.axon_stop_nrt_profile.restype = ctypes.c_int64

    @contextlib.contextmanager
    def _hook(output_dir, device_ids):
        import jax

        jax.devices()
        if device_ids:
            ids = (ctypes.c_int64 * len(device_ids))(*device_ids)
            rc = lib.axon_start_nrt_profile(ids, len(device_ids))
        else:
            rc = lib.axon_start_nrt_profile(None, 0)
        if rc != 0:
            raise RuntimeError(f"axon_start_nrt_profile rc={rc}")
        try:
            yield
        finally:
            n = lib.axon_stop_nrt_profile(str(output_dir).encode())
            print(f"profile: {n} file(s) written to {output_dir}", file=sys.stderr)

    _state = {"hook": _hook}
    mod = types.ModuleType("antenv.axon_hooks")
    mod.get_axon_ntff_profile_hook = lambda: _state["hook"]
    mod.set_axon_ntff_profile_hook = lambda h: _state.update(hook=h)
    sys.modules["antenv.axon_hooks"] = mod
    import antenv

    antenv.axon_hooks = mod

    import concourse.bass_utils as bu

    bu.upload_artifacts = lambda tmpdir: tmpdir
    return True


def _tile_plan(q, tile_cls, strad_r):
    """Per-tile PSUM packing plan.

    q: [3] per-core class quotas.  Returns for each tile t a dict with
      spans: list of (psum_off, bank_off, len) matmul segments, cut at
             PSUM 512 boundaries
      w: total packed width
      segs: for straddle tiles, the two (f0, f1) PSUM ranges whose bf16
            sums (S_b then S_a) the host subtracts from the full total;
            None for pure tiles
    """
    off = [0, int(q[0]), int(q[0] + q[1])]
    q = [int(x) for x in q]
    plans = []
    for t in range(NT):
        c = tile_cls[t]
        if c is not None:
            order = [x for x in range(C) if x != c and q[x] > 0]
        else:
            a, b = strad_r[t][0], strad_r[t][1]
            o = 3 - a - b
            order = [x for x in (b, o, a) if q[x] > 0]
        spans = []
        p = 0
        for x in order:
            boff, blen = off[x], q[x]
            s = 0
            while s < blen:
                take = min(blen - s, CH - (p % CH))
                spans.append((p, boff + s, take))
                p += take
                s += take
        w = p
        if c is not None:
            segs = None
        else:
            a, b = strad_r[t][0], strad_r[t][1]
            o = 3 - a - b
            # psum layout [S_b | S_o | S_a]
            segs = [(0, q[b]), (q[b] + q[o], w)]
        plans.append({"spans": spans, "w": w, "segs": segs})
    return plans


def _build(plans, off_tiles, emit_order, mm_mode):
    import ml_dtypes  # noqa: F401

    in_dt = mybir.dt.float8e4 if mm_mode == "f8" else BF16

    AW = APC + 8               # anchor block: 256 own + 3 gT + 5 pad
    IW = AW + W                # one merged input: [anchors | batch emb]
    NV = 4 * NT + 2 * NOWN + 128

    nc = bacc.Bacc()
    inp_d = nc.declare_dram_parameter("inp", [D, IW], in_dt, isOutput=False)
    bankT_d = nc.declare_dram_parameter("bankT", [D, W], in_dt, isOutput=False)
    vecs_d = nc.declare_dram_parameter("vecs", [128, NV], F32, isOutput=False)
    oout_d = nc.declare_dram_parameter("oout", [128, OC], F32, isOutput=True)

    with tile.TileContext(nc) as tc:
        with (
            tc.tile_pool(name="big", bufs=1) as bigp,
            tc.tile_pool(name="sm", bufs=1) as smp,
            tc.tile_pool(name="scr", bufs=2) as scrp,
            tc.tile_pool(name="i16", bufs=2) as i16p,
            tc.tile_pool(name="ps", bufs=2, space="PSUM") as psp,
        ):
            inp_t = bigp.tile([D, IW], in_dt, tag="inp")
            bank_t = bigp.tile([D, W], in_dt, tag="bankT")

            def own(t):
                return inp_t[:, t * 128:(t + 1) * 128]

            def ecol(a, b):
                return inp_t[:, AW + a:AW + b]
            vecs_t = smp.tile([128, NV], F32, tag="vecs")
            o = [0]

            def vslice(w):
                a = o[0]; o[0] += w
                return vecs_t[:, a:a + w]
            invt_t = vslice(NT)
            ninvt_t = vslice(NT)
            invo_t = vslice(NOWN)
            ninvo_t = vslice(NOWN)
            sA_t = vslice(NT)
            sB_t = vslice(NT)
            eye_t = vslice(128)
            junkx_t = bigp.tile([128, CH], in_dt, tag="junkx")

            oout_t = smp.tile([128, OC], F32, tag="oout")
            eyemul = smp.tile([128, 128], F32, tag="eyemul")
            warm = smp.tile([128, 1], F32, tag="warm")
            sdiag = [smp.tile([128, 1], F32, tag=f"sdiag{t}", name=f"sdiag{t}")
                     for t in range(NOWN)]

            # DMA issues first on each queue so transfers begin ASAP; the
            # exp table load + warm activation follow on the scalar queue
            # and run during the transfer window.  inp is split so the
            # anchor block (prelude operands) lands first.
            H = W // 2
            junkw_t = bigp.tile([128, 128], in_dt, tag="junkw")
            if WARMUP:
                nc.vector.memset(junkw_t[:], 0.0)
                nc.vector.memset(junkx_t[:], 0.0)
            nc.sync.dma_start(out=inp_t[:, 0:AW], in_=inp_d[:, 0:AW])
            nc.scalar.dma_start(out=vecs_t[:], in_=vecs_d[:])
            nc.sync.dma_start(out=inp_t[:, AW:IW], in_=inp_d[:, AW:IW])
            nc.scalar.dma_start(out=bank_t[:], in_=bankT_d[:])

            nc.vector.memset(junkx_t[:, 0:1], 0.0)
            nc.scalar.activation(warm[:], junkx_t[:, 0:1], AF.Exp)

            # PE warmup on garbage operands (HAM clock-gate opener):
            # issued dependency-free right at queue start so the ~3.4us of
            # junk matmuls land inside the DMA window
            if WARMUP:
                warm_ps = psp.tile([128, W], F32, tag="chunk", name="warm_ps")
                for w in range(WARMUP):
                    nc.tensor.matmul(
                        warm_ps[:, (w % 8) * 256:((w % 8) + 1) * 256],
                        junkw_t[:], junkx_t[:, 0:256], start=True, stop=True,
                    )

            # prelude: self-similarity diag + positives row-sums.  Runs
            # first so the selfe exps land in ACT's otherwise-idle DMA-wait
            # window; its DVE consumers finish before bb1 needs the slot
            post_ps = psp.tile([128, W], F32, tag="chunk", name="post_ps")
            for t in range(NOWN):
                nc.tensor.matmul(
                    post_ps[:, t * 128:(t + 1) * 128], own(t), own(t),
                    start=True, stop=True,
                )
            for t in range(NOWN):
                nc.tensor.matmul(
                    post_ps[:, 256 + t * C:256 + (t + 1) * C], own(t),
                    inp_t[:, APC:APC + C], start=True, stop=True,
                )
            for t in range(NOWN):
                nc.vector.tensor_mul(
                    eyemul[:], post_ps[:, t * 128:(t + 1) * 128], eye_t[:])
                nc.vector.reduce_sum(sdiag[t][:], eyemul[:], axis=AX.X)
                nc.scalar.activation(
                    oout_t[:, OC_SELFE + t:OC_SELFE + t + 1], sdiag[t][:],
                    AF.Exp, bias=ninvo_t[:, t:t + 1], scale=invo_t[:, t:t + 1],
                )
                nc.vector.tensor_copy(
                    out=oout_t[:, OC_SDIAG + t:OC_SDIAG + t + 1], in_=sdiag[t][:])
            nc.vector.tensor_copy(
                out=oout_t[:, OC_RAW3:OC_RAW3 + NOWN * C],
                in_=post_ps[:, 256:256 + NOWN * C])

            def emit_bb(t, accum):
                """Full [128, 2048] bb chunk: 4 matmuls + exp w/ accum."""
                ps = psp.tile([128, W], F32, tag="chunk", name=f"bb{t}")
                for qq in range(W // CH):
                    nc.tensor.matmul(
                        ps[:, qq * CH:(qq + 1) * CH], own(t),
                        inp_t[:, AW + qq * CH:AW + (qq + 1) * CH],
                        start=True, stop=True,
                    )
                scr = scrp.tile([128, W], BF16, tag="scr", name=f"bbs{t}")
                sc, bi = invo_t[:, t:t + 1], ninvo_t[:, t:t + 1]
                if isinstance(accum, list):
                    for (a, b, acol) in accum:
                        nc.scalar.activation(
                            scr[:, a:b], ps[:, a:b], AF.Exp,
                            bias=bi, scale=sc,
                            accum_out=oout_t[:, acol:acol + 1])
                else:
                    nc.scalar.activation(
                        scr[:], ps[:], AF.Exp, bias=bi, scale=sc,
                        accum_out=oout_t[:, accum:accum + 1])

            def emit_bank(t, si):
                """Complement-packed bank chunk for anchor tile t.

                si: straddle index (0/1) if tile t straddles, else None.
                Pure ACT: one Exp over [0:w) with accum_out = den col.
                Straddle: Exp over the full packed width, accum = total T;
                  DVE sums the S_b / S_a bf16 ranges; host subtracts.
                DVE path: Schraudolph int16 convert + one bf16 range sum."""
                plan = plans[t]
                w = plan["w"]
                ps = psp.tile([128, W], F32, tag="chunk", name=f"ps{t}")
                lhs = ecol(t * 128, (t + 1) * 128)
                for (poff, boff, ln) in plan["spans"]:
                    nc.tensor.matmul(
                        ps[:, poff:poff + ln], lhs, bank_t[:, boff:boff + ln],
                        start=True, stop=True,
                    )
                if t in off_tiles:
                    i16 = i16p.tile([128, W], I16, tag="i16", name=f"i16{t}")
                    nc.vector.tensor_scalar(
                        out=i16[:, 0:w], in0=ps[:, 0:w],
                        scalar1=sA_t[:, t:t + 1], scalar2=sB_t[:, t:t + 1],
                        op0=ALU.mult, op1=ALU.add)
                    nc.vector.reduce_sum(
                        oout_t[:, OC_DEN + t:OC_DEN + t + 1],
                        i16[:, 0:w].bitcast(BF16), axis=AX.X)
                else:
                    scr = scrp.tile([128, W], BF16, tag="scr", name=f"scr{t}")
                    nc.scalar.activation(
                        scr[:, 0:w], ps[:, 0:w], AF.Exp,
                        bias=ninvt_t[:, t:t + 1], scale=invt_t[:, t:t + 1],
                        accum_out=oout_t[:, OC_DEN + t:OC_DEN + t + 1])
                    if plan["segs"] is not None:
                        for j, (f0, f1) in enumerate(plan["segs"]):
                            nc.vector.reduce_sum(
                                oout_t[:, OC_DEN2 + 2 * si + j:
                                       OC_DEN2 + 2 * si + j + 1],
                                scr[:, f0:f1], axis=AX.X)

            # bb0 opens the stream (needs only the first input push)
            if BB0SPLIT:
                emit_bb(0, [(0, H, OC_BBT + 0), (H, W, OC_BB0B)])
            else:
                emit_bb(0, OC_BBT + 0)

            # bank tiles in planned emission order (off tiles at odd slots)
            strad_seen = 0
            for t in emit_order:
                si = None
                if plans[t]["segs"] is not None:
                    si = strad_seen
                    strad_seen += 1
                emit_bank(t, si)

            # bb1 last: no trailing DVE reduce, so the output DMA fires
            # right after its accumulator read
            emit_bb(1, OC_BBT + 1)

            nc.sync.dma_start(out=oout_d[:], in_=oout_t[:])

    nc.compile()
    return nc


def kernel(embeddings, labels, bank_embs, bank_labels, class_temps):
    global LAST_EXEC_TIME_NS
    import ml_dtypes

    f8 = ml_dtypes.float8_e4m3
    in_np = f8 if MM_MODE == "f8" else ml_dtypes.bfloat16

    emb = np.asarray(embeddings, dtype=np.float32)
    bank = np.asarray(bank_embs, dtype=np.float32)
    lab = np.asarray(labels).astype(np.int64).ravel()
    blab = np.asarray(bank_labels).astype(np.int64).ravel()
    ct = np.asarray(class_temps, dtype=np.float32).ravel()

    # sort batch and bank by class
    bord = np.argsort(lab, kind="stable")
    slab = lab[bord]
    emb_s = emb[bord]                                  # [B, D] f32, sorted
    cnt = np.bincount(lab, minlength=C)
    mord = np.argsort(blab, kind="stable")
    bank_s = bank[mord]
    mcnt = np.bincount(blab, minlength=C)

    # per-core class quotas; <=7*C leftover cols folded in on the host
    q = (mcnt // NCORES).astype(np.int64)              # [3]
    assert int(q.sum()) <= W
    cls_off = np.concatenate([[0], np.cumsum(mcnt)[:-1]])

    # anchor-tile purity (compile-time, same for all cores)
    tile_cls = []
    strad_r = {}
    for t in range(NT):
        seg = slab[t * 128:(t + 1) * 128]
        c_lo, c_hi = int(seg[0]), int(seg[-1])
        if c_lo == c_hi:
            tile_cls.append(c_lo)
        else:
            tile_cls.append(None)
            r = int(np.searchsorted(seg, c_lo, side="right"))
            assert seg[r] == c_hi, "tile straddles >2 classes"
            strad_r[t] = (c_lo, c_hi, r)

    plans = _tile_plan(q, tile_cls, strad_r)

    # quantized operands (shared by device and host-side corrections)
    embq = emb_s.astype(in_np)                         # [B, D]
    bankq = bank_s.astype(in_np)
    embq_f = embq.astype(np.float32)
    bankq_f = bankq.astype(np.float32)
    g = np.stack([emb_s[slab == c].sum(axis=0) for c in range(C)], axis=1)
    gq = g.astype(in_np)                               # [D, 3]

    inv_t_all = (1.0 / ct[slab]).astype(np.float32)    # [B] per sorted anchor

    # DVE-offloaded pure tiles, interleaved at odd emission slots
    pure = [t for t in range(NT) if tile_cls[t] is not None]
    noff = min(NOFF, len(pure), NT // 2)
    off_tiles = (set(pure[int(i)] for i in
                     np.linspace(0, len(pure) - 1, noff).round())
                 if noff > 0 else set())
    on_list = [t for t in range(NT) if t not in off_tiles]
    off_list = [t for t in range(NT) if t in off_tiles]
    emit_order = []
    ia = ib = 0
    for j in range(NT):
        if j % 2 == 1 and ib < len(off_list):
            emit_order.append(off_list[ib]); ib += 1
        else:
            emit_order.append(on_list[ia]); ia += 1

    # per-class Schraudolph bias tuning: pick corr_c that zeroes the mean
    # relative error over the y distribution of this class's logits
    corr_cls = np.zeros(C)
    if off_tiles:
        sgrid = np.linspace(-4.0, 4.0, 4001) / np.sqrt(D)
        wpdf = np.exp(-0.5 * (sgrid * np.sqrt(D)) ** 2)
        for c in range(C):
            it = 1.0 / float(ct[c])
            y = it * (sgrid - 1.0)
            exact = np.exp(y)
            best, bestv = 0.0, np.inf
            for corr in np.linspace(0.0, 12.0, 121):
                i16v = np.clip(np.rint(A16 * y + B16_BASE - corr), 0, 32767)
                approx = i16v.astype(np.int16).view(ml_dtypes.bfloat16).astype(np.float64)
                bias = abs(np.sum(wpdf * approx) / np.sum(wpdf * exact) - 1.0)
                if bias < bestv:
                    best, bestv = corr, bias
            corr_cls[c] = best

    nc = _build(plans, off_tiles, emit_order, MM_MODE)

    eye128 = np.eye(128, dtype=np.float32)
    embT = np.ascontiguousarray(embq.T)                # [D, B], shared
    invt_cols = np.ascontiguousarray(inv_t_all.reshape(NT, 128).T)
    sA_cols = (A16 * invt_cols).astype(np.float32)
    corr_all = corr_cls[slab]
    sB_all = (B16_BASE - corr_all - A16 * inv_t_all.astype(np.float64))
    sB_cols = np.ascontiguousarray(sB_all.reshape(NT, 128).T).astype(np.float32)
    AW = APC + 8
    in_maps = []
    for k in range(NCORES):
        asl = slice(k * APC, (k + 1) * APC)
        inp = np.zeros((D, AW + W), dtype=in_np)
        inp[:, 0:APC] = embq[asl].T
        inp[:, APC:APC + C] = gq
        inp[:, AW:AW + W] = embT
        bankT = np.zeros((D, W), dtype=in_np)
        pos = 0
        for c in range(C):
            sel = bankq[cls_off[c] + k * q[c]: cls_off[c] + (k + 1) * q[c]]
            bankT[:, pos:pos + q[c]] = sel.T
            pos += int(q[c])
        ivo = inv_t_all[asl]
        vparts = [
            invt_cols, -invt_cols,
            np.ascontiguousarray(ivo.reshape(NOWN, 128).T),
            np.ascontiguousarray((-ivo).reshape(NOWN, 128).T),
            sA_cols, sB_cols,
        ]
        vecs = np.concatenate(vparts + [eye128], axis=1).astype(np.float32)
        in_maps.append({
            "inp": np.ascontiguousarray(inp),
            "bankT": np.ascontiguousarray(bankT),
            "vecs": np.ascontiguousarray(vecs),
        })

    trace = os.environ.get("SUPCON_TRACE", "0") == "1"
    if trace:
        trace = _install_trace_shim()
    res = run_bass_kernel_spmd(nc, in_maps, core_ids=list(range(NCORES)), trace=trace)
    LAST_EXEC_TIME_NS = res.exec_time_ns

    # ---- host assembly (fp64) ----
    inv64 = inv_t_all.astype(np.float64)
    den = np.zeros(B, dtype=np.float64)
    raw3_own = np.zeros(B, dtype=np.float64)
    sdiag_own = np.zeros(B, dtype=np.float64)
    tidx = np.arange(128)
    strad_order = sorted(strad_r.keys(), key=lambda t: emit_order.index(t))
    for k in range(NCORES):
        oo = np.asarray(res.results[k]["oout"], dtype=np.float64)  # [128, OC]
        for t in range(NT):
            a_idx = t * 128 + tidx
            if tile_cls[t] is not None or t in off_tiles:
                den[a_idx] += oo[:, OC_DEN + t]
            else:
                # rows [0:r) are class a: den = T - S_a; rows [r:) class b:
                # den = T - S_b (cols OC_DEN2+2si = S_b, +2si+1 = S_a)
                r = strad_r[t][2]
                si = strad_order.index(t)
                T = oo[:, OC_DEN + t]
                s_b = oo[:, OC_DEN2 + 2 * si]
                s_a = oo[:, OC_DEN2 + 2 * si + 1]
                den[a_idx[:r]] += T[:r] - s_a[:r]
                den[a_idx[r:]] += T[r:] - s_b[r:]
        for t in range(NOWN):
            a_idx = k * APC + t * 128 + tidx            # own anchors
            bbT = oo[:, OC_BBT + t]
            if t == 0 and BB0SPLIT:
                bbT = bbT + oo[:, OC_BB0B]
            den[a_idx] += bbT - oo[:, OC_SELFE + t]
            sdiag_own[a_idx] = oo[:, OC_SDIAG + t]
            cls = slab[a_idx]
            raw3_own[a_idx] = oo[tidx, OC_RAW3 + t * 3 + cls]

    # leftover (overflow) bank columns, folded in exactly on the host
    ov_cols, ov_cls = [], []
    for c in range(C):
        lo, hi = cls_off[c] + NCORES * q[c], cls_off[c] + mcnt[c]
        for j in range(lo, hi):
            ov_cols.append(j)
            ov_cls.append(c)
    if ov_cols:
        bq = bankq_f[ov_cols]                           # [n_ov, D]
        s_ov = embq_f @ bq.T                            # [B, n_ov]
        terms = np.exp(inv64[:, None] * (s_ov.astype(np.float64) - 1.0))
        mask = slab[:, None] != np.asarray(ov_cls)[None, :]
        den += (terms * mask).sum(axis=1)

    pos_cnt = (cnt[slab] - 1).astype(np.float64)
    pos_sum = raw3_own - sdiag_own
    pos_mean = pos_sum / np.maximum(pos_cnt, 1.0)
    log_denom = inv64 + np.log(den)
    coef = BASE_TEMP * inv64
    loss_i = coef * (log_denom - pos_mean)
    valid = pos_cnt > 0
    n_valid = int(valid.sum())
    loss = (loss_i * valid).sum() / max(n_valid, 1)
    return np.float32(loss)
